# revision 1
# baseline (speedup 1.0000x reference)
"""Trainium2 Bass kernel for SimCLR-style contrastive loss (B=8192, D=512).

Math (matches reference):
    f_norm = f / ||f||
    sim    = f_norm @ f_norm.T / T
    lse_i  = logsumexp_{j != i} sim_ij
    pos_i  = sim[i, (i + B/2) mod B]
    loss   = mean_i(lse_i - pos_i)

Device strategy (8 cores, data parallel over rows):
  - Host passes each core a row-rotated, transposed bf16 copy of
    features [D, B] so one SPMD program works for every core: core c's
    rotated copy starts at global row c*R, hence its own rows occupy
    local columns [0, R) and row r's positive partner sits at local
    column r + B/2 (never wraps for r < R <= B/2).
  - Loads split across HWDGE (sync) and SWDGE (gpsimd) queues.
  - Normalization: squares on DVE (bf16, 2x mode), column sum-of-
    squares via ones-vector bf16 matmul on PE, Ln staged per chunk
    (phase N uses only the natural_log ACT table set); then per-chunk
    Exp(-0.5*ln) (phase B onward uses only the exp set -> 3 table
    loads total), partition_broadcast on GpSimd to a [128, B] bf16
    tile, normalize ft in place on DVE (bf16 2x mode).
  - Main gram matmul in bf16 (1 cyc/row).  Each [128 x GW] PSUM group
    gets: diagonal killed with a -1e30*I add, partner diagonal
    extracted with tensor_mul against (-1/T)*I + row reduce, then
    fused exp+row-sum on ACT (exp((cos-1)/T), in place in PSUM).
  - Per-row output y_i = ln(sum_i) - pos_cos_i/T returned as [R] f32;
    host computes loss = 1/T + mean(y).
"""

import functools
import sys

sys.path.insert(0, "/opt/trn_rl_repo")

import ml_dtypes
import numpy as np

import concourse.bass as bass
import concourse.mybir as mybir
import concourse.tile as tile
from concourse import bacc
from concourse.bass_utils import run_bass_kernel_spmd
from concourse.tile import add_dep_helper

B = 8192
D = 512
NCORES = 8
R = B // NCORES  # rows per core
TEMP = 0.07
INV_T = 1.0 / TEMP

F32 = mybir.dt.float32
BF16 = mybir.dt.bfloat16
AF = mybir.ActivationFunctionType
ALU = mybir.AluOpType


@functools.lru_cache(maxsize=None)
def build(b=B, d=D, r=R):
    """Build the single-core SPMD program (identical on all cores)."""
    KT = d // 128  # k-tiles over feature dim
    RT = r // 128  # row tiles per core
    CH = 512  # normalization chunk width
    NCH = b // CH
    LW = min(1024, b)  # DMA piece width
    NLW = b // LW
    GW = min(1024, b)  # main-loop PSUM group width (2 banks)
    NG = b // GW

    nc = bacc.Bacc(None, target_bir_lowering=False)
    ftd = nc.dram_tensor("features_t", [d, b], BF16, kind="ExternalInput")
    outd = nc.dram_tensor("losses", [r], F32, kind="ExternalOutput")

    with tile.TileContext(nc) as tc:
        with (
            tc.tile_pool(name="ftp", bufs=1) as ftp,
            tc.tile_pool(name="sing", bufs=1) as sing,
            tc.tile_pool(name="sq", bufs=6) as sqp,
            tc.tile_pool(name="nrm", bufs=3) as nrmp,
            tc.tile_pool(name="dump", bufs=2) as dumpp,
            tc.tile_pool(name="ssq", bufs=2, space="PSUM") as ssqp,
            tc.tile_pool(name="mm", bufs=3, space="PSUM") as mmp,
        ):
            ft = [
                ftp.tile([128, b], BF16, tag=f"ft{k}", name=f"ft{k}")
                for k in range(KT)
            ]

            # Pre-place the one ACT table set that covers every function
            # used below (exp, ln, copy): natural_log_exp_and_others.  The
            # Bacc fixpoint pass then inserts no further table loads, so
            # interleaving 1/norm math with main-loop exps costs nothing.
            _ln_exp_set = 6  # index in act_info.json act_func_sets
            _tl = mybir.InstLoadActFuncSet(
                name=nc.get_next_instruction_name(),
                act_func_set_id=_ln_exp_set,
                ins=[],
                outs=[],
            )
            nc.scalar.add_instruction(_tl)

            # Constant tiles.  negI/eye in bf16 so the diagonal poison can
            # be applied by the PE itself (an accumulating I.T @ (-1e30*I)
            # matmul) instead of a DVE pass over PSUM.
            negI = sing.tile([128, 128], BF16)
            nc.gpsimd.memset(negI[:], 0.0)
            nc.gpsimd.affine_select(
                out=negI[:],
                in_=negI[:],
                compare_op=ALU.not_equal,
                fill=-1e30,
                base=0,
                pattern=[[-1, 128]],
                channel_multiplier=1,
            )
            eyeb = sing.tile([128, 128], BF16)
            nc.gpsimd.memset(eyeb[:], 0.0)
            nc.gpsimd.affine_select(
                out=eyeb[:],
                in_=eyeb[:],
                compare_op=ALU.not_equal,
                fill=1.0,
                base=0,
                pattern=[[-1, 128]],
                channel_multiplier=1,
            )
            # (-1/T) * I -- multiplying the partner block by this and row-
            # reducing yields -pos_logit directly.
            negTI = sing.tile([128, 128], F32)
            nc.gpsimd.memset(negTI[:], 0.0)
            nc.gpsimd.affine_select(
                out=negTI[:],
                in_=negTI[:],
                compare_op=ALU.not_equal,
                fill=-INV_T,
                base=0,
                pattern=[[-1, 128]],
                channel_multiplier=1,
            )
            ones_k = sing.tile([128, 1], BF16)
            nc.vector.memset(ones_k[:], 1.0)
            negC = sing.tile([128, 1], F32)
            nc.vector.memset(negC[:], -INV_T)

            inv_row = sing.tile([1, b], BF16)  # 1/norm per column
            bc_sb = sing.tile([128, b], BF16)  # broadcast 1/norm
            spart = sing.tile([128, RT, NG], F32)
            posn = sing.tile([128, RT], F32)
            ysb = sing.tile([128, RT], F32)
            ssum = sing.tile([128, RT], F32)
            lnS = sing.tile([128, RT], F32)

            # ---- loads: sync+scalar HWDGE queues (~170 GB/s each); piece
            # issuance is interleaved with the compute loop so the scalar
            # engine's issue slots don't delay the first 1/norm ACT ops ----
            dma_engines = [nc.sync, nc.scalar]

            def load_pieces(plist, engines):
                for p in plist:
                    ls = slice(p * LW, (p + 1) * LW)
                    for k in range(KT):
                        eng = engines[(p * KT + k) % len(engines)]
                        eng.dma_start(
                            out=ft[k][:, ls],
                            in_=ftd[k * 128 : (k + 1) * 128, ls],
                        )

            # Early pieces split across both HWDGE queues; the later pieces'
            # scalar-side issues are deferred past the prologue so they do
            # not sit ahead of the first Ln/Exp in ACT program order.
            load_pieces(range(min(4, NLW)), dma_engines)
            _later = list(range(4, NLW))

            # ---- normalization, split so the sum-of-squares half (which
            # gates PE/ACT progress) runs well ahead of the in-place scale
            # half (needed only one group before use). ----
            def sumsq_chunk(ch):
                cs = slice(ch * CH, (ch + 1) * CH)
                ssq = ssqp.tile([1, CH], F32, name=f"ssq{ch}", tag="ssq")
                for k in range(KT):
                    sq = sqp.tile([128, CH], BF16, name=f"sq{ch}_{k}", tag="sq")
                    nc.vector.tensor_mul(sq[:], ft[k][:, cs], ft[k][:, cs])
                    nc.tensor.matmul(
                        ssq[:],
                        ones_k[:],
                        sq[:],
                        start=(k == 0),
                        stop=(k == KT - 1),
                    )
                lns = nrmp.tile([1, CH], F32, name=f"lns{ch}", tag="lns")
                nc.scalar.activation(out=lns[:], in_=ssq[:], func=AF.Ln)
                nc.scalar.activation(
                    out=inv_row[0:1, cs], in_=lns[:], func=AF.Exp, scale=-0.5
                )
                nc.gpsimd.partition_broadcast(bc_sb[:, cs], inv_row[0:1, cs])

            def scale_chunk(ch):
                cs = slice(ch * CH, (ch + 1) * CH)
                for k in range(KT):
                    nc.vector.tensor_mul(ft[k][:, cs], ft[k][:, cs], bc_sb[:, cs])

            CPG = GW // CH  # chunks per group
            for ch in range(min(2 * CPG, NCH)):
                sumsq_chunk(ch)
            load_pieces(_later, dma_engines)
            for ch in range(CPG):
                scale_chunk(ch)
            main_groups = []
            for g in range(NG):
                g0 = g * GW
                for ch in range((g + 1) * CPG, (g + 2) * CPG):
                    if ch < NCH:
                        scale_chunk(ch)
                for t in range(RT):
                    ps = mmp.tile([128, GW], F32, tag="mm")
                    for k in range(KT):
                        for n2 in range(GW // 512):
                            nc.tensor.matmul(
                                ps[:, n2 * 512 : (n2 + 1) * 512],
                                ft[k][:, t * 128 : (t + 1) * 128],
                                ft[k][:, g0 + n2 * 512 : g0 + (n2 + 1) * 512],
                                start=(k == 0),
                                stop=(k == KT - 1),
                            )
                    # Positive-pair diagonal: global col b/2 + t*128.
                    pcol = b // 2 + t * 128
                    if g0 <= pcol < g0 + GW:
                        off = pcol - g0
                        dmp = dumpp.tile([128, 128], F32)
                        nc.vector.tensor_mul(dmp[:], ps[:, off : off + 128], negTI[:])
                        nc.vector.tensor_reduce(
                            out=posn[:, t : t + 1],
                            in_=dmp[:],
                            axis=mybir.AxisListType.X,
                            op=ALU.add,
                        )
                    # Self-similarity diagonal: global col t*128 -> -1e30,
                    # applied on the PE as one extra accumulating matmul.
                    dcol = t * 128
                    if g0 <= dcol < g0 + GW:
                        off = dcol - g0
                        nc.tensor.matmul(
                            ps[:, off : off + 128],
                            eyeb[:],
                            negI[:],
                            start=False,
                            stop=True,
                            skip_group_check=True,
                        )
                    # exp((cos - 1)/T) in place + fused row-sum.
                    nc.scalar.activation(
                        out=ps[:],
                        in_=ps[:],
                        func=AF.Exp,
                        scale=INV_T,
                        bias=negC[:],
                        accum_out=spart[:, t, g : g + 1],
                    )
                # Lookahead sum-of-squares AFTER this group's matmuls, so
                # PE's in-order queue never gates a group on a later DMA.
                for ch in range((g + 2) * CPG, (g + 3) * CPG):
                    if ch < NCH:
                        sumsq_chunk(ch)

            # ---- epilogue: y = ln(S) - pos/T ----
            for t in range(RT):
                nc.vector.tensor_reduce(
                    out=ssum[:, t : t + 1],
                    in_=spart[:, t, :],
                    axis=mybir.AxisListType.X,
                    op=ALU.add,
                )
            nc.scalar.activation(out=lnS[:, :RT], in_=ssum[:, :RT], func=AF.Ln)
            nc.vector.tensor_add(ysb[:, :RT], lnS[:, :RT], posn[:, :RT])
            nc.sync.dma_start(
                out=outd[:].rearrange("(t p) -> p t", p=128), in_=ysb[:, :RT]
            )

    nc.finalize()
    return nc


def run(features, b=B, d=D, ncores=NCORES, **kwargs):
    """Run the SPMD kernel; returns (losses[b] fp32, BassKernelResults)."""
    r = b // ncores
    nc = build(b, d, r)
    feats = np.ascontiguousarray(np.asarray(features, dtype=np.float32))
    in_maps = []
    for c in range(ncores):
        rot = np.roll(feats, -c * r, axis=0)
        in_maps.append(
            {"features_t": np.ascontiguousarray(rot.T).astype(ml_dtypes.bfloat16)}
        )
    res = run_bass_kernel_spmd(nc, in_maps, core_ids=list(range(ncores)), **kwargs)
    y = np.concatenate([res.results[c]["losses"] for c in range(ncores)])
    return y, res


def kernel(features):
    y, _ = run(features)
    loss = INV_T + float(np.mean(y.astype(np.float64)))
    return np.float32(loss)



# revision 4
# speedup vs baseline: 1.1497x; 1.1497x over previous
"""Trainium2 Bass kernel for SimCLR-style contrastive loss (B=8192, D=512).

Math (matches reference):
    f_norm = f / ||f||
    sim    = f_norm @ f_norm.T / T
    lse_i  = logsumexp_{j != i} sim_ij
    pos_i  = sim[i, (i + B/2) mod B]
    loss   = mean_i(lse_i - pos_i)

Device strategy (8 cores, data parallel over rows), fp8 edition:
  - Host passes each core a row-rotated, transposed fp8(e4m3) copy of
    the RAW features [D, B]; rotation by c*R makes one SPMD program
    work for every core (own rows at local cols [0, R), positive
    partner of row r at local col r + B/2).
  - Feature k-tiles stored pairwise ([128, 2, B] fp8) so every PE pass
    runs in DoubleRow perf mode (256-deep contraction per pass).
  - Norms: squares on DVE (fp8 out), column sum-of-squares via a
    DoubleRow ones-matmul, Ln/Exp chain on ACT exactly as before.
  - Gram matmul in fp8 DoubleRow; diagonal poisoned by a bf16
    eye @ (-1e30 I) accumulating matmul; partner diagonal extracted
    from f32 PSUM with a (-1/T) I tensor_mul + row reduce on DVE;
    fused exp((cos-1)/T) + row-sum on ACT in place in PSUM.
  - Per-row output y_i = ln(sum_i) - pos_cos_i/T returned as [R] f32;
    host computes loss = 1/T + mean(y).
"""

import functools
import sys

sys.path.insert(0, "/opt/trn_rl_repo")

import ml_dtypes
import numpy as np

import concourse.bass as bass
import concourse.mybir as mybir
import concourse.tile as tile
from concourse import bacc
from concourse.bass_utils import run_bass_kernel_spmd
from concourse.tile import add_dep_helper

B = 8192
D = 512
NCORES = 8
R = B // NCORES  # rows per core
TEMP = 0.07
INV_T = 1.0 / TEMP

F32 = mybir.dt.float32
BF16 = mybir.dt.bfloat16
FP8 = mybir.dt.float8e4
AF = mybir.ActivationFunctionType
ALU = mybir.AluOpType
DR = mybir.MatmulPerfMode.DoubleRow


@functools.lru_cache(maxsize=None)
def build(b=B, d=D, r=R):
    """Build the single-core SPMD program (identical on all cores)."""
    KT = d // 128  # k-tiles over feature dim
    KP = KT // 2  # k-tile PAIRS (DoubleRow contracts 256 at a time)
    RT = r // 128  # row tiles per core
    CH = 512  # normalization chunk width
    NCH = b // CH
    LW = min(1024, b)  # DMA piece width
    NLW = b // LW
    GW = min(1024, b)  # main-loop PSUM group width (2 banks)
    NG = b // GW

    nc = bacc.Bacc(None, target_bir_lowering=False)
    ftd = nc.dram_tensor("features_t", [d, b], FP8, kind="ExternalInput")
    outd = nc.dram_tensor("losses", [r], F32, kind="ExternalOutput")

    with tile.TileContext(nc) as tc:
        with (
            tc.tile_pool(name="ftp", bufs=1) as ftp,
            tc.tile_pool(name="sing", bufs=1) as sing,
            tc.tile_pool(name="sq", bufs=6) as sqp,
            tc.tile_pool(name="nrm", bufs=3) as nrmp,
            tc.tile_pool(name="dump", bufs=2) as dumpp,
            tc.tile_pool(name="ssq", bufs=2, space="PSUM") as ssqp,
            tc.tile_pool(name="mm", bufs=3, space="PSUM") as mmp,
        ):
            # k-tile pairs: ft[p] holds k-tiles 2p (slot 0) and 2p+1 (slot 1)
            ft = [
                ftp.tile([128, 2, b], FP8, tag=f"ft{p}", name=f"ft{p}")
                for p in range(KP)
            ]

            # Pre-place the one ACT table set that covers every function
            # used below (exp, ln, copy): natural_log_exp_and_others.
            _ln_exp_set = 6  # index in act_info.json act_func_sets
            _tl = mybir.InstLoadActFuncSet(
                name=nc.get_next_instruction_name(),
                act_func_set_id=_ln_exp_set,
                ins=[],
                outs=[],
            )
            nc.scalar.add_instruction(_tl)

            # Constant tiles.  negI/eye in bf16 so the diagonal poison can
            # be applied by the PE itself (an accumulating I.T @ (-1e30*I)
            # matmul) instead of a DVE pass over PSUM.
            negI = sing.tile([128, 128], BF16)
            nc.gpsimd.memset(negI[:], 0.0)
            nc.gpsimd.affine_select(
                out=negI[:],
                in_=negI[:],
                compare_op=ALU.not_equal,
                fill=-1e30,
                base=0,
                pattern=[[-1, 128]],
                channel_multiplier=1,
            )
            eyeb = sing.tile([128, 128], BF16)
            nc.gpsimd.memset(eyeb[:], 0.0)
            nc.gpsimd.affine_select(
                out=eyeb[:],
                in_=eyeb[:],
                compare_op=ALU.not_equal,
                fill=1.0,
                base=0,
                pattern=[[-1, 128]],
                channel_multiplier=1,
            )
            # (-1/T) * I -- multiplying the partner block by this and row-
            # reducing yields -pos_logit directly.
            negTI = sing.tile([128, 128], F32)
            nc.gpsimd.memset(negTI[:], 0.0)
            nc.gpsimd.affine_select(
                out=negTI[:],
                in_=negTI[:],
                compare_op=ALU.not_equal,
                fill=-INV_T,
                base=0,
                pattern=[[-1, 128]],
                channel_multiplier=1,
            )
            ones1 = sing.tile([128, 1], FP8)
            nc.vector.memset(ones1[:], 1.0)
            negC = sing.tile([128, 1], F32)
            nc.vector.memset(negC[:], -INV_T)

            inv_row = sing.tile([1, b], BF16)  # 1/norm per column
            bc_sb = sing.tile([128, b], BF16)  # broadcast 1/norm
            spart = sing.tile([128, RT, NG], F32)
            posn = sing.tile([128, RT], F32)
            ysb = sing.tile([128, RT], F32)
            ssum = sing.tile([128, RT], F32)
            lnS = sing.tile([128, RT], F32)

            # ---- loads: sync+scalar HWDGE queues; piece issuance is
            # interleaved with the compute loop ----
            dma_engines = [nc.sync, nc.scalar]

            def load_pieces(plist, engines):
                for p in plist:
                    ls = slice(p * LW, (p + 1) * LW)
                    for k in range(KT):
                        eng = engines[(p * KT + k) % len(engines)]
                        eng.dma_start(
                            out=ft[k // 2][:, k % 2, ls],
                            in_=ftd[k * 128 : (k + 1) * 128, ls],
                        )

            load_pieces(range(min(4, NLW)), dma_engines)
            _later = list(range(4, NLW))

            # ---- normalization ----
            def sumsq_chunk(ch):
                cs = slice(ch * CH, (ch + 1) * CH)
                ssq = ssqp.tile([1, CH], F32, name=f"ssq{ch}", tag="ssq")
                for p in range(KP):
                    sq = sqp.tile([128, 2, CH], FP8, name=f"sq{ch}_{p}", tag="sq")
                    nc.vector.tensor_mul(sq[:], ft[p][:, :, cs], ft[p][:, :, cs])
                    for kk in range(2):
                        # dual-fp8 ldweights rejects M=1 stationaries, so the
                        # ones-matmul runs in regular (non-DoubleRow) mode.
                        nc.tensor.matmul(
                            ssq[:],
                            ones1[:],
                            sq[:, kk, :],
                            start=(p == 0 and kk == 0),
                            stop=(p == KP - 1 and kk == 1),
                        )
                lns = nrmp.tile([1, CH], F32, name=f"lns{ch}", tag="lns")
                nc.scalar.activation(out=lns[:], in_=ssq[:], func=AF.Ln)
                nc.scalar.activation(
                    out=inv_row[0:1, cs], in_=lns[:], func=AF.Exp, scale=-0.5
                )
                nc.gpsimd.partition_broadcast(bc_sb[:, cs], inv_row[0:1, cs])

            def scale_chunk(ch):
                cs = slice(ch * CH, (ch + 1) * CH)
                for p in range(KP):
                    for kk in range(2):
                        nc.vector.tensor_mul(
                            ft[p][:, kk, cs], ft[p][:, kk, cs], bc_sb[:, cs]
                        )

            CPG = GW // CH  # chunks per group
            for ch in range(min(2 * CPG, NCH)):
                sumsq_chunk(ch)
            load_pieces(_later, dma_engines)
            for ch in range(CPG):
                scale_chunk(ch)
            for g in range(NG):
                g0 = g * GW
                for ch in range((g + 1) * CPG, (g + 2) * CPG):
                    if ch < NCH:
                        scale_chunk(ch)
                for t in range(RT):
                    ps = mmp.tile([128, GW], F32, tag="mm")
                    for p in range(KP):
                        for n2 in range(GW // 512):
                            nc.tensor.matmul(
                                ps[:, n2 * 512 : (n2 + 1) * 512],
                                ft[p][:, :, t * 128 : (t + 1) * 128],
                                ft[p][:, :, g0 + n2 * 512 : g0 + (n2 + 1) * 512],
                                start=(p == 0),
                                stop=(p == KP - 1),
                                perf_mode=DR,
                            )
                    # Positive-pair diagonal: global col b/2 + t*128.
                    pcol = b // 2 + t * 128
                    if g0 <= pcol < g0 + GW:
                        off = pcol - g0
                        dmp = dumpp.tile([128, 128], F32)
                        nc.vector.tensor_mul(dmp[:], ps[:, off : off + 128], negTI[:])
                        nc.vector.tensor_reduce(
                            out=posn[:, t : t + 1],
                            in_=dmp[:],
                            axis=mybir.AxisListType.X,
                            op=ALU.add,
                        )
                    # Self-similarity diagonal: global col t*128 -> -1e30,
                    # applied on the PE as one extra accumulating matmul.
                    dcol = t * 128
                    if g0 <= dcol < g0 + GW:
                        off = dcol - g0
                        nc.tensor.matmul(
                            ps[:, off : off + 128],
                            eyeb[:],
                            negI[:],
                            start=False,
                            stop=True,
                            skip_group_check=True,
                        )
                    # exp((cos - 1)/T) in place + fused row-sum.
                    nc.scalar.activation(
                        out=ps[:],
                        in_=ps[:],
                        func=AF.Exp,
                        scale=INV_T,
                        bias=negC[:],
                        accum_out=spart[:, t, g : g + 1],
                    )
                # Lookahead sum-of-squares AFTER this group's matmuls, so
                # PE's in-order queue never gates a group on a later DMA.
                for ch in range((g + 2) * CPG, (g + 3) * CPG):
                    if ch < NCH:
                        sumsq_chunk(ch)

            # ---- epilogue: y = ln(S) - pos/T ----
            for t in range(RT):
                nc.vector.tensor_reduce(
                    out=ssum[:, t : t + 1],
                    in_=spart[:, t, :],
                    axis=mybir.AxisListType.X,
                    op=ALU.add,
                )
            nc.scalar.activation(out=lnS[:, :RT], in_=ssum[:, :RT], func=AF.Ln)
            nc.vector.tensor_add(ysb[:, :RT], lnS[:, :RT], posn[:, :RT])
            nc.sync.dma_start(
                out=outd[:].rearrange("(t p) -> p t", p=128), in_=ysb[:, :RT]
            )

    nc.finalize()
    return nc


def run(features, b=B, d=D, ncores=NCORES, **kwargs):
    """Run the SPMD kernel; returns (losses[b] fp32, BassKernelResults)."""
    r = b // ncores
    nc = build(b, d, r)
    feats = np.ascontiguousarray(np.asarray(features, dtype=np.float32))
    in_maps = []
    for c in range(ncores):
        rot = np.roll(feats, -c * r, axis=0)
        in_maps.append(
            {
                "features_t": np.ascontiguousarray(rot.T).astype(
                    ml_dtypes.float8_e4m3
                )
            }
        )
    res = run_bass_kernel_spmd(nc, in_maps, core_ids=list(range(ncores)), **kwargs)
    y = np.concatenate([res.results[c]["losses"] for c in range(ncores)])
    return y, res


def kernel(features):
    y, _ = run(features)
    loss = INV_T + float(np.mean(y.astype(np.float64)))
    return np.float32(loss)


# revision 6
# speedup vs baseline: 1.6133x; 1.4033x over previous
"""Trainium2 Bass kernel for SimCLR-style contrastive loss (B=8192, D=512).

Symmetric fp8 edition: sim = fn fn^T is symmetric, so each computed entry
E_ij = exp((cos_ij-1)/T + C) can credit BOTH row-sum S_i (via the ACT
accumulator) and col-sum S_j (via a PE ones-matmul over the fp8 E tile).
Each core therefore computes only 9/16 of its [1024, 8192] slab:

  local cols [0, 1024)     diag block (own rows)      row-sums only
  local cols [1024, 4096)  offset-1..3 blocks         row + col sums
  local cols [4096, 5120)  antipodal half-block       row + col sums
      row-tiles 0-3 x cols [4096, 4608), row-tiles 4-7 x cols [4608, 5120)

The host hands each core a row-permuted fp8 copy of the raw features
(transposed, only the 5120 needed columns); for cores 4-7 the two
antipodal half-slabs are swapped so one SPMD program covers every core.
The positive-pair diagonal lives in the computed antipodal quadrants of
cores 0-3; cores 4-7 reuse those values by symmetry on the host.

Per-core outputs: rsums[1024] (ACT accum), csums[4096] (PE ones-matmul
col sums, DMA'd straight out of PSUM), posn[1024] (-pos/T, garbage on
cores 4-7).  Host scatter-adds S, then loss = 1/T - C + mean(ln S + posn).

E is stored shifted by C=14 so its [0.01, 55] range survives the fp8 cast
used by the col-sum matmuls.
"""

import functools
import sys

sys.path.insert(0, "/opt/trn_rl_repo")

import ml_dtypes
import numpy as np

import concourse.bass as bass
import concourse.mybir as mybir
import concourse.tile as tile
from concourse import bacc
from concourse.bass_utils import run_bass_kernel_spmd

B = 8192
D = 512
NCORES = 8
R = B // NCORES  # rows per core
NCOL = B // 2 + R  # columns computed per core
TEMP = 0.07
INV_T = 1.0 / TEMP
C_SHIFT = 14.0

F32 = mybir.dt.float32
BF16 = mybir.dt.bfloat16
FP8 = mybir.dt.float8e4
AF = mybir.ActivationFunctionType
ALU = mybir.AluOpType
DR = mybir.MatmulPerfMode.DoubleRow

KT = D // 128  # 4 k-tiles
KP = KT // 2  # 2 k-tile pairs (DoubleRow)
RT = R // 128  # 8 row tiles
CH = 512  # normalization chunk width
NCH = NCOL // CH  # 10
LW = 1024  # DMA piece width
NLW = NCOL // LW  # 5
GW = 1024  # main PSUM group width (2 banks)
NMG = 4  # main column groups (cols [0, 4096))
NSLOT = NMG + 1  # spart accumulation slots (+ antipodal)


@functools.lru_cache(maxsize=None)
def build():
    nc = bacc.Bacc(None, target_bir_lowering=False)
    ftd = nc.dram_tensor("features_t", [D, NCOL], FP8, kind="ExternalInput")
    outr = nc.dram_tensor("rsums", [R], F32, kind="ExternalOutput")
    outp = nc.dram_tensor("posn", [R], F32, kind="ExternalOutput")
    outc = nc.dram_tensor("csums", [NCOL - R], F32, kind="ExternalOutput")

    with tile.TileContext(nc) as tc:
        with (
            tc.tile_pool(name="ftp", bufs=1) as ftp,
            tc.tile_pool(name="ep", bufs=1) as epool,
            tc.tile_pool(name="sing", bufs=1) as sing,
            tc.tile_pool(name="sq", bufs=6) as sqp,
            tc.tile_pool(name="nrm", bufs=3) as nrmp,
            tc.tile_pool(name="dump", bufs=2) as dumpp,
            tc.tile_pool(name="ssq", bufs=2, space="PSUM") as ssqp,
            tc.tile_pool(name="cs", bufs=2, space="PSUM") as csp,
            tc.tile_pool(name="mm", bufs=2, space="PSUM") as mmp,
        ):
            ft = [
                ftp.tile([128, 2, NCOL], FP8, tag=f"ft{p}", name=f"ft{p}")
                for p in range(KP)
            ]
            E13 = epool.tile([128, RT, (NMG - 1) * GW], FP8)
            Eap = epool.tile([128, RT, 512], FP8)

            _tl = mybir.InstLoadActFuncSet(
                name=nc.get_next_instruction_name(),
                act_func_set_id=6,  # natural_log_exp_and_others
                ins=[],
                outs=[],
            )
            nc.scalar.add_instruction(_tl)

            # Constants
            negI = sing.tile([128, 128], BF16)
            nc.gpsimd.memset(negI[:], 0.0)
            nc.gpsimd.affine_select(
                out=negI[:], in_=negI[:], compare_op=ALU.not_equal,
                fill=-1e30, base=0, pattern=[[-1, 128]], channel_multiplier=1,
            )
            eyeb = sing.tile([128, 128], BF16)
            nc.gpsimd.memset(eyeb[:], 0.0)
            nc.gpsimd.affine_select(
                out=eyeb[:], in_=eyeb[:], compare_op=ALU.not_equal,
                fill=1.0, base=0, pattern=[[-1, 128]], channel_multiplier=1,
            )
            negTI = sing.tile([128, 128], F32)
            nc.gpsimd.memset(negTI[:], 0.0)
            nc.gpsimd.affine_select(
                out=negTI[:], in_=negTI[:], compare_op=ALU.not_equal,
                fill=-INV_T, base=0, pattern=[[-1, 128]], channel_multiplier=1,
            )
            ones_dr = sing.tile([128, 2, 16], FP8)
            nc.vector.memset(ones_dr[:], 1.0)
            biasC = sing.tile([128, 1], F32)
            nc.vector.memset(biasC[:], C_SHIFT - INV_T)

            inv_row = sing.tile([1, NCOL], BF16)
            bc_sb = sing.tile([128, NCOL], BF16)
            spart = sing.tile([128, RT, NSLOT], F32)
            posn = sing.tile([128, RT], F32)
            ssum = sing.tile([128, RT], F32)
            csum_sb = sing.tile([1, NCOL - R], F32)

            dma_engines = [nc.sync, nc.scalar]

            def load_pieces(plist):
                for p in plist:
                    ls = slice(p * LW, (p + 1) * LW)
                    for k in range(KT):
                        eng = dma_engines[(p * KT + k) % 2]
                        eng.dma_start(
                            out=ft[k // 2][:, k % 2, ls],
                            in_=ftd[k * 128 : (k + 1) * 128, ls],
                        )

            load_pieces(range(4))

            def sumsq_chunk(ch):
                cs = slice(ch * CH, (ch + 1) * CH)
                ssq = ssqp.tile([16, CH], F32, name=f"ssq{ch}", tag="ssq")
                for p in range(KP):
                    sq = sqp.tile([128, 2, CH], FP8, name=f"sq{ch}_{p}", tag="sq")
                    nc.vector.tensor_mul(sq[:], ft[p][:, :, cs], ft[p][:, :, cs])
                    nc.tensor.matmul(
                        ssq[:], ones_dr[:], sq[:],
                        start=(p == 0), stop=(p == KP - 1), perf_mode=DR,
                    )
                lns = nrmp.tile([1, CH], F32, name=f"lns{ch}", tag="lns")
                nc.scalar.activation(out=lns[:], in_=ssq[0:1, :], func=AF.Ln)
                nc.scalar.activation(
                    out=inv_row[0:1, cs], in_=lns[:], func=AF.Exp, scale=-0.5
                )
                nc.gpsimd.partition_broadcast(bc_sb[:, cs], inv_row[0:1, cs])

            def scale_chunk(ch):
                cs = slice(ch * CH, (ch + 1) * CH)
                for p in range(KP):
                    for kk in range(2):
                        nc.vector.tensor_mul(
                            ft[p][:, kk, cs], ft[p][:, kk, cs], bc_sb[:, cs]
                        )

            def colsum(lo, width, rhs_tile, rhs_lo, upairs, *, tag):
                """Column sums of E over row-tile pairs -> DMA to outc.

                lo: local column (>= R) of the first summed column;
                rhs_tile[:, 2u:2u+2, rhs_lo:rhs_lo+width] are the E slabs.
                """
                cps = csp.tile([16, width], F32, name=f"cs{tag}", tag="cs")
                for i, u in enumerate(upairs):
                    nc.tensor.matmul(
                        cps[:],
                        ones_dr[:],
                        rhs_tile[:, 2 * u : 2 * u + 2, rhs_lo : rhs_lo + width],
                        start=(i == 0),
                        stop=(i == len(upairs) - 1),
                        perf_mode=DR,
                    )
                nc.vector.tensor_copy(
                    out=csum_sb[0:1, lo - R : lo - R + width], in_=cps[0:1, :]
                )

            # Prologue: chunks for g0+g1, scale g0 (also the stationaries).
            for ch in range(4):
                sumsq_chunk(ch)
            load_pieces([4])
            for ch in range(2):
                scale_chunk(ch)

            for g in range(NMG):
                for ch in range(2 * (g + 1), 2 * (g + 2)):
                    if ch < NCH:
                        scale_chunk(ch)
                g0 = g * GW
                for t in range(RT):
                    ps = mmp.tile([128, GW], F32, tag="mm")
                    for p in range(KP):
                        for n2 in range(GW // 512):
                            nc.tensor.matmul(
                                ps[:, n2 * 512 : (n2 + 1) * 512],
                                ft[p][:, :, t * 128 : (t + 1) * 128],
                                ft[p][:, :, g0 + n2 * 512 : g0 + (n2 + 1) * 512],
                                start=(p == 0),
                                stop=(p == KP - 1),
                                perf_mode=DR,
                            )
                    if g == 0:
                        off = t * 128
                        nc.tensor.matmul(
                            ps[:, off : off + 128],
                            eyeb[:],
                            negI[:],
                            start=False,
                            stop=True,
                            skip_group_check=True,
                        )
                        nc.scalar.activation(
                            out=ps[:], in_=ps[:], func=AF.Exp,
                            scale=INV_T, bias=biasC[:],
                            accum_out=spart[:, t, g : g + 1],
                        )
                    else:
                        nc.scalar.activation(
                            out=E13[:, t, (g - 1) * GW : g * GW],
                            in_=ps[:], func=AF.Exp,
                            scale=INV_T, bias=biasC[:],
                            accum_out=spart[:, t, g : g + 1],
                        )
                for ch in range(2 * (g + 2), 2 * (g + 3)):
                    if ch < NCH:
                        sumsq_chunk(ch)
                if g >= 2:
                    gp = g - 1  # colsum one group behind
                    for s2 in range(2):
                        colsum(
                            gp * GW + s2 * 512, 512, E13,
                            (gp - 1) * GW + s2 * 512,
                            range(4), tag=f"g{gp}s{s2}",
                        )

            # Antipodal half-block: row-tiles 0-3 x [4096, 4608),
            # row-tiles 4-7 x [4608, 5120).  pos diag at (r, 4096+r).
            for t in range(RT):
                aoff = 4096 + (0 if t < 4 else 512)
                ps = mmp.tile([128, GW], F32, tag="mm", name=f"aps{t}")
                for p in range(KP):
                    nc.tensor.matmul(
                        ps[:, 0:512],
                        ft[p][:, :, t * 128 : (t + 1) * 128],
                        ft[p][:, :, aoff : aoff + 512],
                        start=(p == 0),
                        stop=(p == KP - 1),
                        perf_mode=DR,
                    )
                off = (t % 4) * 128
                dmp = dumpp.tile([128, 128], F32, name=f"dmp{t}")
                nc.vector.tensor_mul(dmp[:], ps[:, off : off + 128], negTI[:])
                nc.vector.tensor_reduce(
                    out=posn[:, t : t + 1],
                    in_=dmp[:],
                    axis=mybir.AxisListType.X,
                    op=ALU.add,
                )
                nc.scalar.activation(
                    out=Eap[:, t, :], in_=ps[:, 0:512], func=AF.Exp,
                    scale=INV_T, bias=biasC[:],
                    accum_out=spart[:, t, NMG : NMG + 1],
                )
            # colsum for main group 3, then the antipodal slabs.
            for s2 in range(2):
                colsum(3 * GW + s2 * 512, 512, E13, 2 * GW + s2 * 512,
                       range(4), tag=f"g3s{s2}")
            colsum(4096, 512, Eap, 0, [0, 1], tag="apA")
            colsum(4608, 512, Eap, 0, [2, 3], tag="apB")

            # Epilogue: per-row partial sums + pos out.
            for t in range(RT):
                nc.vector.tensor_reduce(
                    out=ssum[:, t : t + 1],
                    in_=spart[:, t, :],
                    axis=mybir.AxisListType.X,
                    op=ALU.add,
                )
            nc.sync.dma_start(
                out=outr[:].rearrange("(t p) -> p t", p=128), in_=ssum[:, :RT]
            )
            nc.scalar.dma_start(
                out=outp[:].rearrange("(t p) -> p t", p=128), in_=posn[:, :RT]
            )
            nc.sync.dma_start(out=outc[:], in_=csum_sb[0:1, :])

    nc.finalize()
    return nc


def core_perm(c):
    """Global row index for each local column of core c."""
    perm = (np.arange(B) + c * R) % B
    if c >= NCORES // 2:
        tmp = perm[4096:4608].copy()
        perm[4096:4608] = perm[4608:5120]
        perm[4608:5120] = tmp
    return perm[:NCOL]


def make_in_map(feats, c):
    perm = core_perm(c)
    return {
        "features_t": np.ascontiguousarray(feats[perm].T).astype(
            ml_dtypes.float8_e4m3
        )
    }


def run(features, **kwargs):
    """Run the SPMD kernel; returns (y[b] fp32 per-row losses-ish, results).

    y_i = ln(S_i) - C_SHIFT - pos_i/T, so loss = 1/T + mean(y).
    """
    nc = build()
    feats = np.ascontiguousarray(np.asarray(features, dtype=np.float32))
    in_maps = [make_in_map(feats, c) for c in range(NCORES)]
    res = run_bass_kernel_spmd(nc, in_maps, core_ids=list(range(NCORES)), **kwargs)
    S = np.zeros(B, dtype=np.float64)
    POS = np.zeros(B, dtype=np.float64)
    for c in range(NCORES):
        perm = core_perm(c)
        S[perm[:R]] += res.results[c]["rsums"].astype(np.float64)
        S[perm[R:NCOL]] += res.results[c]["csums"].astype(np.float64)
        if c < NCORES // 2:
            pp = res.results[c]["posn"].astype(np.float64)
            POS[perm[:R]] = pp
            POS[(perm[:R] + B // 2) % B] = pp
    y = np.log(S) - C_SHIFT + POS
    return y.astype(np.float32), res


def kernel(features):
    y, _ = run(features)
    loss = INV_T + float(np.mean(y.astype(np.float64)))
    return np.float32(loss)
